# revision 23
# baseline (speedup 1.0000x reference)
"""Trainium2 Bass kernel for nn_MultiHeadAttention (dense transformer block:
qkv proj + RoPE + causal SDPA + out proj), tensor-parallel over (batch, heads)
across 8 NeuronCores.

Sharding: 2 batches x 16 heads = 32 (b,h) pairs; core c handles batch c//4,
heads 4*(c%4)..4*(c%4)+3. Each core computes qkv for its 4 heads (from the
full x of its batch), RoPE, causal attention, and a PARTIAL output
projection (its heads' rows of Wproj); the host sums the 4 partials per
batch.

Projections (qkv, out) run in fp8-e4m3 DoubleRow mode with 3-term error
compensation: each operand X is split host-side (or on-device for Y) into
X8 = fp8(X) and Xr = fp8(X - X8); the product X@W is computed as
X8@W8 + Xr@W8 + X8@Wr (the Xr@Wr term, ~0.07%, is dropped). DoubleRow
packs two 128-contraction products per matmul at 0.5 cycles/row, so the
3-term scheme costs 0.75x bf16 cycles at ~bf16 accuracy. Weights are
pre-scaled x32 so their fp8 residuals stay out of the subnormal floor;
the 1/32 is folded into the RoPE tables / copy-out activation scales.

Attention stays bf16. The softmax denominator is NOT computed with a
full-rate ones-matmul; instead each P^T tile is used as matmul weights
against a [128,1] ones column (ap_size=1 => ~free), giving l per q-chunk
as a PSUM column, which is PE-transposed, reciprocal'd on DVE, and
broadcast back across partitions with tiny selector matmuls.

Layout notes:
- x is passed pre-transposed per batch (xT [D, S]) so the contraction dim
  (model dim) lands on SBUF partitions with no on-device transpose.
- q/k head dims are permuted host-side into a 16-interleaved (even,odd)
  order so RoPE's pair swap is a quadrant-local DVE stream_shuffle.
  Attention scores are invariant to this (q and k permuted identically).
- Scores are computed transposed (S^T [kv, q]); exp() runs without
  max-subtraction: |scores| < ~10 for this input distribution.
"""
import sys

sys.path.insert(0, "/opt/trn_rl_repo")

import numpy as np
import ml_dtypes

import concourse.bass as bass
import concourse.mybir as mybir
import concourse.tile as tile

P = 128
B, S, D = 2, 2048, 2048
NH, HD = 16, 128
NH_CORE = 4  # heads per core
HCOLS = NH_CORE * HD  # 512
KT = D // P  # 16 k-tiles
TT = S // P  # 16 token tiles
QC = 512  # q-chunk width
NQC = S // QC  # 4
ROPE_THETA = 10000.0
SCALE = HD**-0.5
NEG = -30000.0
WS = 32.0  # fp8 weight pre-scale

F32 = mybir.dt.float32
BF16 = mybir.dt.bfloat16
FP8 = mybir.dt.float8e4
DR = mybir.MatmulPerfMode.DoubleRow
E4NP = ml_dtypes.float8_e4m3

_SWAP16 = [(i + 16) % 32 for i in range(32)]


# ---------------------------------------------------------------------------
# host-side constant tables
# ---------------------------------------------------------------------------
def _dim_perm():
    """Permutation p -> original head-dim index, 16-interleaved even/odd."""
    perm = np.zeros(HD, dtype=np.int64)
    for p in range(HD):
        qd, sl = p // 32, p % 32
        i = 16 * qd + (sl % 16)
        perm[p] = 2 * i if sl < 16 else 2 * i + 1
    return perm


def _rope_tables():
    """ctab[p,t], stab[p,t] (sign-baked) for the permuted head-dim layout."""
    perm = _dim_perm()
    inv_freq = 1.0 / (ROPE_THETA ** (np.arange(0, HD, 2, dtype=np.float64) / HD))
    t = np.arange(S, dtype=np.float64)
    ctab = np.zeros((HD, S), dtype=np.float64)
    stab = np.zeros((HD, S), dtype=np.float64)
    for p in range(HD):
        qd, sl = p // 32, p % 32
        i = 16 * qd + (sl % 16)
        ang = t * inv_freq[i]
        ctab[p] = np.cos(ang)
        stab[p] = -np.sin(ang) if sl < 16 else np.sin(ang)
    return ctab.astype(np.float32), stab.astype(np.float32)


def _tri_mask():
    """[P, P] f32: 0 where kv(row) <= q(col) else NEG."""
    b = np.arange(P)[:, None]
    a = np.arange(P)[None, :]
    return np.where(b <= a, 0.0, NEG).astype(np.float32)


def _sel_mat():
    """[4, 4*P] bf16: sel[:, s*P:(s+1)*P] is all-zero except row s = ones."""
    m = np.zeros((4, 4 * P), dtype=np.float32)
    for s in range(4):
        m[s, s * P : (s + 1) * P] = 1.0
    return m.astype(ml_dtypes.bfloat16)


def _split8(a):
    """fp8 hi/lo split: returns (a8, ar) as float8_e4m3 arrays."""
    a = np.asarray(a, np.float32)
    a8 = a.astype(E4NP)
    ar = (a - a8.astype(np.float32)).astype(E4NP)
    return a8, ar


# ---------------------------------------------------------------------------
# device kernel
# ---------------------------------------------------------------------------
def _build_nc():
    nc = bass.Bass()

    # slot-major layouts: x slots (xr, x8); W slots (W8, Wr); Y slots (Yr, Y8)
    xp = nc.declare_dram_parameter("xp", [2 * D, S], FP8, isOutput=False)
    Wq = nc.declare_dram_parameter("Wq", [2 * D, HCOLS], FP8, isOutput=False)
    Wk = nc.declare_dram_parameter("Wk", [2 * D, HCOLS], FP8, isOutput=False)
    Wv = nc.declare_dram_parameter("Wv", [2 * D, HCOLS], FP8, isOutput=False)
    Wp = nc.declare_dram_parameter("Wp", [2 * HCOLS, D], FP8, isOutput=False)
    out = nc.declare_dram_parameter("out", [S, D], BF16, isOutput=True)

    # sqrt(SCALE)/WS on both q and k tables => scores scaled by SCALE and
    # the x32 weight pre-scale undone
    ctab_np, stab_np = _rope_tables()
    rt = np.float32(np.sqrt(SCALE) / WS)
    cq_d = nc.inline_tensor((ctab_np * rt).astype(ml_dtypes.bfloat16), "cq")
    sq_d = nc.inline_tensor((stab_np * rt).astype(ml_dtypes.bfloat16), "sq")
    mask_d = nc.inline_tensor(
        _tri_mask().astype(ml_dtypes.bfloat16), "trimask"
    )
    sel_d = nc.inline_tensor(_sel_mat(), "selmat")
    ident_d = nc.inline_tensor(np.eye(P, dtype=np.float32), "ident")
    identb_d = nc.inline_tensor(
        np.eye(P, dtype=np.float32).astype(ml_dtypes.bfloat16), "identb"
    )

    xp_t = xp[:].rearrange("(two ko p) t -> p two ko t", p=P, two=2)
    Wq_t = Wq[:].rearrange("(two ko p) m -> p two ko m", p=P, two=2)
    Wk_t = Wk[:].rearrange("(two ko p) m -> p two ko m", p=P, two=2)
    Wv_t = Wv[:].rearrange("(two ko p) m -> p two ko m", p=P, two=2)
    Wp_t = Wp[:].rearrange("(two ho p) n -> p two ho n", p=P, two=2)
    out_t = out[:].rearrange("(to p) n -> p to n", p=P)

    def mm3(ps, W_sb, x_sb, wcols, xcols, w_stationary=True):
        """3-term fp8 DoubleRow accumulation over the KT contraction tiles.

        W_sb: [P, 2, KT, *] with slots (W8, Wr); x_sb likewise with slots
        (xr, x8). wcols/xcols are free-dim slices. The stationary operand is
        W when w_stationary else x. Main terms (x8*W8, needing only the
        earliest-DMA'd slots) are issued before the cross terms.
        """
        nmm = KT // 2 + KT
        i = 0
        # main terms: k-pairs of x8 (slot 1) against W8 (slot 0)
        for ki in range(0, KT, 2):
            a = W_sb[:, 0, ki : ki + 2, wcols]
            b = x_sb[:, 1, ki : ki + 2, xcols]
            lhsT, rhs = (a, b) if w_stationary else (b, a)
            nc.tensor.matmul(ps, lhsT, rhs, start=(i == 0), stop=(i == nmm - 1),
                             perf_mode=DR)
            i += 1
        # cross terms: (W8, Wr) x (xr, x8)
        for ki in range(KT):
            a = W_sb[:, :, ki, wcols]
            b = x_sb[:, :, ki, xcols]
            lhsT, rhs = (a, b) if w_stationary else (b, a)
            nc.tensor.matmul(ps, lhsT, rhs, start=(i == 0), stop=(i == nmm - 1),
                             perf_mode=DR)
            i += 1

    with tile.TileContext(nc) as tc:
        with (
            tc.tile_pool(name="persist", bufs=1) as pp,
            tc.tile_pool(name="work", bufs=2) as wk,
        ):
            # persistent tiles
            cq = pp.tile([P, S], BF16)
            sq = pp.tile([P, S], BF16)
            trimask = pp.tile([P, P], BF16)
            identb = pp.tile([P, P], BF16)
            sel = pp.tile([4, 4 * P], BF16)
            ident = pp.tile([P, P], F32)
            ones_col = pp.tile([P, 1], BF16)
            nc.vector.memset(ones_col, 1.0)
            ones_row = pp.tile([1, P], BF16)
            nc.vector.memset(ones_row, 1.0)
            zrow = pp.tile([1, QC], BF16)
            nc.vector.memset(zrow, 0.0)

            Qt = pp.tile([P, NH_CORE, S], BF16)
            Kt = pp.tile([P, NH_CORE, S], BF16)
            Vt = pp.tile([P, TT, HCOLS], BF16)
            Yp = pp.tile([P, 2, NH_CORE, S], FP8)  # slots (Yr, Y8)

            # ------- phase 1: q/k projection + RoPE, V tiles 0..3 ----------
            # xp and Wv persist into phase 2: V tiles 4..15 are produced
            # there, interleaved into the attention stream as PE filler.
            with tc.tile_pool(name="mm1b", bufs=1) as mm1b:
                xp_sb = mm1b.tile([P, 2, KT, S], FP8)
                Wv_sb = mm1b.tile([P, 2, KT, HCOLS], FP8)

                def v_tile_mms(tt, ps):
                    """Returns the 24 matmul thunks of V tile tt (token-major;
                    x stationary), in mm3 order (mains then crosses)."""
                    tsl = slice(tt * P, (tt + 1) * P)
                    thunks = []
                    nmm = KT // 2 + KT
                    i = [0]
                    def mk(lhsT, rhs):
                        j = i[0]
                        thunks.append(lambda: nc.tensor.matmul(
                            ps, lhsT, rhs, start=(j == 0), stop=(j == nmm - 1),
                            perf_mode=DR))
                        i[0] += 1
                    for ki in range(0, KT, 2):
                        mk(xp_sb[:, 1, ki : ki + 2, tsl],
                           Wv_sb[:, 0, ki : ki + 2, :])
                    for ki in range(KT):
                        mk(xp_sb[:, :, ki, tsl], Wv_sb[:, :, ki, :])
                    return thunks

                with (
                    tc.tile_pool(name="mm1a", bufs=1) as mm1a,
                    tc.tile_pool(name="ps_mm1", bufs=8, space="PSUM") as psA,
                ):
                    Wq_sb = mm1a.tile([P, 2, KT, HCOLS], FP8)
                    Wk_sb = mm1a.tile([P, 2, KT, HCOLS], FP8)
                    # DMA priority: x8 + Wq feed the first matmuls, then xr
                    # (cross terms), then Wk, then Wv
                    for ki in range(KT):
                        nc.sync.dma_start(xp_sb[:, 1, ki], xp_t[:, 1, ki])
                        nc.gpsimd.dma_start(Wq_sb[:, 0, ki], Wq_t[:, 0, ki])
                        nc.gpsimd.dma_start(Wq_sb[:, 1, ki], Wq_t[:, 1, ki])
                    # RoPE tables: must land before the first q-tile's
                    # DVE chain (~18us) to avoid backpressure on psum bufs
                    nc.gpsimd.dma_start(cq, cq_d[:])
                    nc.gpsimd.dma_start(sq, sq_d[:])
                    for ki in range(KT):
                        nc.sync.dma_start(xp_sb[:, 0, ki], xp_t[:, 0, ki])
                        nc.gpsimd.dma_start(Wk_sb[:, 0, ki], Wk_t[:, 0, ki])
                        nc.gpsimd.dma_start(Wk_sb[:, 1, ki], Wk_t[:, 1, ki])
                    for ki in range(KT):
                        nc.gpsimd.dma_start(Wv_sb[:, 0, ki], Wv_t[:, 0, ki])
                        nc.gpsimd.dma_start(Wv_sb[:, 1, ki], Wv_t[:, 1, ki])
                    # phase-2 constants: not needed until attention
                    nc.gpsimd.dma_start(trimask, mask_d[:])
                    nc.gpsimd.dma_start(identb, identb_d[:])
                    nc.gpsimd.dma_start(sel, sel_d[:])
                    nc.gpsimd.dma_start(ident, ident_d[:])

                    # q and k projections with fused RoPE
                    for W_sb, O_t in ((Wq_sb, Qt), (Wk_sb, Kt)):
                        for h in range(NH_CORE):
                            for tcx in range(NQC):
                                ps = psA.tile([P, QC], F32, tag="ps")
                                mm3(ps, W_sb, xp_sb,
                                    slice(h * HD, (h + 1) * HD),
                                    slice(tcx * QC, (tcx + 1) * QC))
                                csl = cq[:, tcx * QC : (tcx + 1) * QC]
                                ssl = sq[:, tcx * QC : (tcx + 1) * QC]
                                pc = wk.tile([P, QC], BF16, tag="pc")
                                nc.scalar.activation(
                                    pc, ps, mybir.ActivationFunctionType.Copy
                                )
                                xsw = wk.tile([P, QC], BF16, tag="xsw")
                                nc.vector.stream_shuffle(xsw, pc, _SWAP16)
                                m1 = wk.tile([P, QC], BF16, tag="m1")
                                nc.vector.tensor_mul(m1, pc, csl)
                                m2 = wk.tile([P, QC], BF16, tag="m2")
                                nc.vector.tensor_mul(m2, xsw, ssl)
                                nc.vector.tensor_add(
                                    O_t[:, h, tcx * QC : (tcx + 1) * QC], m1, m2
                                )

                    # V tiles 0..3 (needed by the first attention q-chunk)
                    for tt in range(4):
                        ps = psA.tile([P, HCOLS], F32, tag="ps")
                        for th in v_tile_mms(tt, ps):
                            th()
                        nc.scalar.activation(
                            Vt[:, tt], ps, mybir.ActivationFunctionType.Copy,
                            scale=1.0 / WS,
                        )

                # ------ phase 2: attention + V tiles 4..15 + out proj -------
                with (
                    tc.tile_pool(name="attn", bufs=1) as atp,
                    tc.tile_pool(name="outp", bufs=4) as outp,
                    tc.tile_pool(name="pt", bufs=3) as ptp,
                    tc.tile_pool(name="ps_s", bufs=2, space="PSUM") as psS,
                    tc.tile_pool(name="ps_o", bufs=2, space="PSUM") as psO,
                    tc.tile_pool(name="ps_m", bufs=2, space="PSUM") as psM,
                    tc.tile_pool(name="ps_b", bufs=1, space="PSUM") as psB,
                    tc.tile_pool(name="ps_p", bufs=1, space="PSUM") as psP,
                ):
                    Wp_sb = atp.tile([P, 2, NH_CORE, D], FP8)
                    for s in range(2):
                        for ho in range(NH_CORE):
                            nc.sync.dma_start(Wp_sb[:, s, ho], Wp_t[:, s, ho])

                    out_n = [0]

                    def out_group(tt, ncx, pool, tag="psp"):
                        """One (tt, ncx) out-projection psum group: 6
                        DoubleRow matmuls + copy-out + DMA."""
                        ps = pool.tile([P, QC], F32, tag=tag,
                                       name=f"og{tt}_{ncx}")
                        i = 0
                        for ho in range(0, NH_CORE, 2):
                            nc.tensor.matmul(
                                ps,
                                Yp[:, 1, ho : ho + 2, tt * P : (tt + 1) * P],
                                Wp_sb[:, 0, ho : ho + 2,
                                      ncx * QC : (ncx + 1) * QC],
                                start=(i == 0),
                                stop=False,
                                perf_mode=DR,
                            )
                            i += 1
                        for ho in range(NH_CORE):
                            nc.tensor.matmul(
                                ps,
                                Yp[:, :, ho, tt * P : (tt + 1) * P],
                                Wp_sb[:, :, ho, ncx * QC : (ncx + 1) * QC],
                                start=False,
                                stop=(ho == NH_CORE - 1),
                                perf_mode=DR,
                            )
                        obp = outp.tile([P, QC], BF16, tag="obp")
                        # alternate the psum->sbuf copy between ACT and DVE
                        # to keep both below the PE roofline
                        if out_n[0] % 2 == 0:
                            nc.scalar.activation(
                                obp, ps, mybir.ActivationFunctionType.Copy
                            )
                        else:
                            nc.vector.tensor_copy(obp, ps)
                        out_n[0] += 1
                        nc.sync.dma_start(
                            out_t[:, tt, ncx * QC : (ncx + 1) * QC], obp
                        )

                    # filler fifo: 4-matmul chunks of V tiles 4..15, then
                    # out-projection groups as Y chunks complete
                    fifo = []
                    vps = {}

                    def enqueue_v(tt):
                        # psum tile created lazily at the first chunk so psP's
                        # buffer isn't claimed before the previous user's
                        # reads are issued
                        def first_chunk(thunks=None, tt=tt):
                            ps = psP.tile([P, HCOLS], F32, tag="psp",
                                          name=f"vps{tt}")
                            vps[tt] = ps
                            vthunks[tt] = v_tile_mms(tt, ps)
                            for th in vthunks[tt][0:4]:
                                th()
                        fifo.append(("v", tt, 0, first_chunk))
                        for j in range(4, 24, 4):
                            fifo.append(("v", tt, j, None))

                    vthunks = {}

                    def drain2(k, tail=False):
                        n = 0
                        while n < k and fifo:
                            item = fifo.pop(0)
                            if item[0] == "v":
                                _, tt, j, fc = item
                                if fc is not None:
                                    fc()
                                else:
                                    for th in vthunks[tt][j : j + 4]:
                                        th()
                                if j == 20:  # last chunk: copy out Vt
                                    nc.scalar.activation(
                                        Vt[:, tt],
                                        vps.pop(tt),
                                        mybir.ActivationFunctionType.Copy,
                                        scale=1.0 / WS,
                                    )
                                    del vthunks[tt]
                            else:
                                if tail and n % 2 == 1:
                                    out_group(item[1], item[2], psB, tag="bb")
                                else:
                                    out_group(item[1], item[2], psP)
                            n += 1

                    def av_step(h, qc, njb, o_ps, lm, jb, off, pt):
                        d = jb - 4 * qc
                        nc.tensor.matmul(
                            o_ps[:, off:],
                            Vt[:, jb, h * HD : (h + 1) * HD],
                            pt[:, off:],
                            start=(jb == 0),
                            stop=(jb == njb - 1),
                        )
                        # denominator: P^T tile as weights against a ones
                        # column (ap_size=1 => ~free)
                        for sub in range(max(d, 0), 4):
                            nc.tensor.matmul(
                                lm[:, sub : sub + 1],
                                pt[:, sub * P : (sub + 1) * P],
                                ones_col,
                                start=False,
                                stop=(jb == 4 * qc + sub),
                                skip_group_check=True,
                            )

                    pending = None  # deferred normalization epilogue

                    def epilogue(h, qc, o_ps, lm):
                        # l [q,4sub] -> transpose -> 1/l -> broadcast [128,q]
                        l_sb = wk.tile([P, 4], F32, tag="lsb")
                        nc.vector.tensor_copy(l_sb, lm[:, 0:4])
                        nc.tensor.transpose(lm[0:4, 4 : 4 + P], l_sb, ident)
                        rb = wk.tile([4, P], BF16, tag="rb")
                        with nc.allow_low_precision(
                            reason="softmax denom reciprocal to bf16"
                        ):
                            nc.vector.reciprocal(rb, lm[0:4, 4 : 4 + P])
                        bb = psB.tile([P, QC], F32, tag="bb")
                        # explicit zero then accumulate (safe under bank- or
                        # byte-granular psum zeroing)
                        nc.tensor.matmul(bb, ones_row, zrow, start=True,
                                         stop=False, skip_group_check=True)
                        for sub in range(4):
                            nc.tensor.matmul(
                                bb[:, sub * P : (sub + 1) * P],
                                sel[:, sub * P : (sub + 1) * P],
                                rb,
                                start=False,
                                stop=(sub == 3),
                                skip_group_check=True,
                            )
                        ym = wk.tile([P, QC], BF16, tag="ym")
                        nc.vector.tensor_mul(ym, o_ps, bb)
                        y8 = Yp[:, 1, h, qc * QC : (qc + 1) * QC]
                        nc.vector.tensor_copy(y8, ym)
                        nc.vector.tensor_sub(
                            Yp[:, 0, h, qc * QC : (qc + 1) * QC], ym, y8
                        )

                    for qc in range(NQC):
                        if qc < NQC - 1:
                            for tt in range(4 * qc + 4, 4 * qc + 8):
                                enqueue_v(tt)
                        for h in range(NH_CORE):
                            o_ps = psO.tile([P, QC], F32, tag="ops")
                            lm = psM.tile([P, QC], F32, tag="lm")
                            # zero the l columns explicitly
                            nc.tensor.matmul(lm[:, 0:4], ones_row,
                                             zrow[:, 0:4], start=True,
                                             stop=False,
                                             skip_group_check=True)
                            njb = 4 * qc + 4
                            prev = None  # software-pipelined AV/l step
                            for jb in range(njb):
                                d = jb - 4 * qc  # diag offset if >= 0
                                off = P * d if d > 0 else 0
                                # pace the filler: early q-chunks must push V
                                # tiles through before they're consumed; the
                                # last q-chunk must stretch its few out-proj
                                # groups across 64 steps
                                if qc == NQC - 1:
                                    if (4 * h + jb) % 4 == 0:
                                        drain2(1)
                                else:
                                    drain2(2 if len(fifo) > 12 else 1)
                                s_ps = psS.tile([P, QC], F32, tag="sps")
                                nc.tensor.matmul(
                                    s_ps[:, off:],
                                    Kt[:, h, jb * P : (jb + 1) * P],
                                    Qt[:, h, qc * QC + off : (qc + 1) * QC],
                                    start=True,
                                    stop=(d < 0),
                                    skip_group_check=True,
                                )
                                if d >= 0:
                                    # causal mask via PE: += I^T @ trimask
                                    nc.tensor.matmul(
                                        s_ps[:, off : off + P],
                                        identb,
                                        trimask,
                                        start=False,
                                        stop=True,
                                        skip_group_check=True,
                                    )
                                pt = ptp.tile([P, QC], BF16, tag="pt")
                                nc.scalar.activation(
                                    pt[:, off:],
                                    s_ps[:, off:],
                                    mybir.ActivationFunctionType.Exp,
                                )
                                if prev is not None:
                                    av_step(h, qc, njb, o_ps, lm, *prev)
                                prev = (jb, off, pt)
                            av_step(h, qc, njb, o_ps, lm, *prev)
                            if pending is not None:
                                epilogue(*pending)
                                ph, pqc = pending[0], pending[1]
                                if ph == NH_CORE - 1:
                                    for tt in range(4 * pqc, 4 * pqc + 4):
                                        for ncx in range(D // QC):
                                            fifo.append(("o", tt, ncx))
                            pending = (h, qc, o_ps, lm)
                    epilogue(*pending)
                    for tt in range(4 * (NQC - 1), TT):
                        for ncx in range(D // QC):
                            fifo.append(("o", tt, ncx))
                    # tail: alternate psum banks (psB is free after the final
                    # epilogue) to pipeline group vs copy
                    drain2(len(fifo), tail=True)
    return nc


# ---------------------------------------------------------------------------
# legalization: this walrus build supports only ONE sync wait per instruction
# ---------------------------------------------------------------------------
_ENGINE_SEM_PREFIX = {
    "PE": "PE_",
    "DVE": "DVE_",
    "ACT": "ACT_",
    "Pool": "POOL_",
    "SP": "SP_",
}
_wf_counter = [0]


def _legalize(nc, max_waits=1):
    for f in nc.m.functions:
        for bb in f.blocks:
            new_insts = []
            for inst in bb.instructions:
                si = getattr(inst, "sync_info", None)
                eng = getattr(inst, "engine", None)
                if si is None or not si.on_wait or eng is None:
                    new_insts.append(inst)
                    continue
                waits = list(si.on_wait)
                pref = _ENGINE_SEM_PREFIX.get(eng.name)
                if pref is not None:
                    waits = [
                        w
                        for w in waits
                        if not (
                            w.sync_type == "semaphore"
                            and w.ant_name.startswith(pref)
                        )
                    ]
                if len(waits) > max_waits:
                    for w in waits[:-max_waits]:
                        _wf_counter[0] += 1
                        nop = mybir.InstNoOp(
                            name=f"I-waitfix-{_wf_counter[0]}", ins=[], outs=[]
                        )
                        nop.engine = eng
                        nop.sync_info = mybir.SyncInfo(on_wait=[w], on_update=[])
                        new_insts.append(nop)
                    waits = waits[-max_waits:]
                if len(waits) != len(si.on_wait):
                    inst.sync_info = mybir.SyncInfo(
                        on_wait=waits, on_update=list(si.on_update)
                    )
                new_insts.append(inst)
            bb.instructions[:] = new_insts


# ---------------------------------------------------------------------------
# SPMD runner (mirrors concourse.bass2jax.run_bass_via_pjrt, kept resident)
# ---------------------------------------------------------------------------
class _Runner:
    def __init__(self, nc, n_cores=8):
        import jax
        from jax.sharding import Mesh, PartitionSpec
        from jax.experimental.shard_map import shard_map
        from concourse import bass2jax
        from concourse.bass2jax import _bass_exec_p, install_neuronx_cc_hook

        install_neuronx_cc_hook()
        self.jax = jax
        self.nc = nc
        self.n_cores = n_cores
        partition_name = (
            nc.partition_id_tensor.name if nc.partition_id_tensor else None
        )
        in_names, out_names, out_avals, zero_outs = [], [], [], []
        for alloc in nc.m.functions[0].allocations:
            if not isinstance(alloc, mybir.MemoryLocationSet):
                continue
            name = alloc.memorylocations[0].name
            if alloc.kind == "ExternalInput":
                if name != partition_name:
                    in_names.append(name)
            elif alloc.kind == "ExternalOutput":
                shape = tuple(alloc.tensor_shape)
                dtype = mybir.dt.np(alloc.dtype)
                out_names.append(name)
                out_avals.append(jax.core.ShapedArray(shape, dtype))
                zero_outs.append(np.zeros(shape, dtype))
        self.in_names, self.out_names = in_names, out_names
        self.out_avals, self.zero_outs = out_avals, zero_outs
        n_params, n_outs = len(in_names), len(out_names)
        all_in_names = in_names + out_names
        if partition_name is not None:
            all_in_names.append(partition_name)
        donate = tuple(range(n_params, n_params + n_outs))

        def _body(*args):
            operands = list(args)
            if partition_name is not None:
                operands.append(bass2jax.partition_id_tensor())
            return tuple(
                _bass_exec_p.bind(
                    *operands,
                    out_avals=tuple(out_avals),
                    in_names=tuple(all_in_names),
                    out_names=tuple(out_names),
                    lowering_input_output_aliases=(),
                    sim_require_finite=True,
                    sim_require_nnan=True,
                    nc=nc,
                )
            )

        devices = jax.devices()[:n_cores]
        mesh = Mesh(np.asarray(devices), ("core",))
        in_specs = (PartitionSpec("core"),) * (n_params + n_outs)
        out_specs = (PartitionSpec("core"),) * n_outs
        self.fn = jax.jit(
            shard_map(
                _body,
                mesh=mesh,
                in_specs=in_specs,
                out_specs=out_specs,
                check_rep=False,
            ),
            donate_argnums=donate,
            keep_unused=True,
        )

    def run(self, in_maps):
        n = self.n_cores
        concat_in = [
            np.concatenate(
                [np.asarray(in_maps[c][name]) for c in range(n)], axis=0
            )
            for name in self.in_names
        ]
        zeros = [
            np.zeros((n * z.shape[0], *z.shape[1:]), z.dtype)
            for z in self.zero_outs
        ]
        out_arrs = self.fn(*concat_in, *zeros)
        return [
            {
                name: np.asarray(out_arrs[i]).reshape(
                    n, *self.out_avals[i].shape
                )[c]
                for i, name in enumerate(self.out_names)
            }
            for c in range(n)
        ]


_RUNNER = None


def _get_runner():
    global _RUNNER
    if _RUNNER is None:
        nc = _build_nc()
        _legalize(nc)
        _RUNNER = _Runner(nc, 8)
    return _RUNNER


# ---------------------------------------------------------------------------
# input prep (host-side sharding + fp8 decomposition)
# ---------------------------------------------------------------------------
def _prep_in_maps(x, Wqkv, Wproj):
    x = np.asarray(x, dtype=np.float32)
    Wqkv = np.asarray(Wqkv, dtype=np.float32)
    Wproj = np.asarray(Wproj, dtype=np.float32)
    perm = _dim_perm()

    xps = []
    for b in range(B):
        xT = np.ascontiguousarray(x[b].T)
        x8, xr = _split8(xT)
        # [2*D, S] slot-major: slot 0 = xr, slot 1 = x8
        xps.append(np.concatenate([xr, x8], axis=0))

    def wsplit(Wm):  # [K, N] -> [2*K, N] slot-major (W8, Wr)
        w8, wr = _split8(Wm * WS)
        return np.concatenate([w8, wr], axis=0)

    in_maps = []
    for c in range(8):
        b, g = c // 4, c % 4
        heads = range(NH_CORE * g, NH_CORE * (g + 1))
        qcols = np.concatenate([h * HD + perm for h in heads])
        in_maps.append(
            {
                "xp": xps[b],
                "Wq": wsplit(Wqkv[:, 0 * D + qcols]),
                "Wk": wsplit(Wqkv[:, 1 * D + qcols]),
                "Wv": wsplit(
                    Wqkv[:, 2 * D + g * HCOLS : 2 * D + (g + 1) * HCOLS]
                ),
                "Wp": wsplit(Wproj[g * HCOLS : (g + 1) * HCOLS, :]),
            }
        )
    return in_maps


# ---------------------------------------------------------------------------
# public entry point
# ---------------------------------------------------------------------------
def kernel(x, Wqkv, Wproj):
    in_maps = _prep_in_maps(x, Wqkv, Wproj)
    results = _get_runner().run(in_maps)
    out = np.zeros((B, S, D), dtype=np.float32)
    for c in range(8):
        out[c // 4] += results[c]["out"].astype(np.float32)
    out *= np.float32(1.0 / WS)  # undo the Wp x32 pre-scale (device keeps x32)
    return out


# revision 24
# speedup vs baseline: 1.0113x; 1.0113x over previous
"""Trainium2 Bass kernel for nn_MultiHeadAttention (dense transformer block:
qkv proj + RoPE + causal SDPA + out proj), tensor-parallel over (batch, heads)
across 8 NeuronCores.

Sharding: 2 batches x 16 heads = 32 (b,h) pairs; core c handles batch c//4,
heads 4*(c%4)..4*(c%4)+3. Each core computes qkv for its 4 heads (from the
full x of its batch), RoPE, causal attention, and a PARTIAL output
projection (its heads' rows of Wproj); the host sums the 4 partials per
batch.

Projections (qkv, out) run in fp8-e4m3 DoubleRow mode with 3-term error
compensation: each operand X is split host-side (or on-device for Y) into
X8 = fp8(X) and Xr = fp8(X - X8); the product X@W is computed as
X8@W8 + Xr@W8 + X8@Wr (the Xr@Wr term, ~0.07%, is dropped). DoubleRow
packs two 128-contraction products per matmul at 0.5 cycles/row, so the
3-term scheme costs 0.75x bf16 cycles at ~bf16 accuracy. Weights are
pre-scaled x32 so their fp8 residuals stay out of the subnormal floor;
the 1/32 is folded into the RoPE tables / copy-out activation scales.

Attention stays bf16. The softmax denominator is NOT computed with a
full-rate ones-matmul; instead each P^T tile is used as matmul weights
against a [128,1] ones column (ap_size=1 => ~free), giving l per q-chunk
as a PSUM column, which is PE-transposed, reciprocal'd on DVE, and
broadcast back across partitions with tiny selector matmuls.

Layout notes:
- x is passed pre-transposed per batch (xT [D, S]) so the contraction dim
  (model dim) lands on SBUF partitions with no on-device transpose.
- q/k head dims are permuted host-side into a 16-interleaved (even,odd)
  order so RoPE's pair swap is a quadrant-local DVE stream_shuffle.
  Attention scores are invariant to this (q and k permuted identically).
- Scores are computed transposed (S^T [kv, q]); exp() runs without
  max-subtraction: |scores| < ~10 for this input distribution.
"""
import sys

sys.path.insert(0, "/opt/trn_rl_repo")

import numpy as np
import ml_dtypes

import concourse.bass as bass
import concourse.mybir as mybir
import concourse.tile as tile

P = 128
B, S, D = 2, 2048, 2048
NH, HD = 16, 128
NH_CORE = 4  # heads per core
HCOLS = NH_CORE * HD  # 512
KT = D // P  # 16 k-tiles
TT = S // P  # 16 token tiles
QC = 512  # q-chunk width
NQC = S // QC  # 4
ROPE_THETA = 10000.0
SCALE = HD**-0.5
NEG = -30000.0
WS = 32.0  # fp8 weight pre-scale

F32 = mybir.dt.float32
BF16 = mybir.dt.bfloat16
FP8 = mybir.dt.float8e4
DR = mybir.MatmulPerfMode.DoubleRow
E4NP = ml_dtypes.float8_e4m3

_SWAP16 = [(i + 16) % 32 for i in range(32)]


# ---------------------------------------------------------------------------
# host-side constant tables
# ---------------------------------------------------------------------------
def _dim_perm():
    """Permutation p -> original head-dim index, 16-interleaved even/odd."""
    perm = np.zeros(HD, dtype=np.int64)
    for p in range(HD):
        qd, sl = p // 32, p % 32
        i = 16 * qd + (sl % 16)
        perm[p] = 2 * i if sl < 16 else 2 * i + 1
    return perm


def _rope_tables():
    """ctab[p,t], stab[p,t] (sign-baked) for the permuted head-dim layout."""
    perm = _dim_perm()
    inv_freq = 1.0 / (ROPE_THETA ** (np.arange(0, HD, 2, dtype=np.float64) / HD))
    t = np.arange(S, dtype=np.float64)
    ctab = np.zeros((HD, S), dtype=np.float64)
    stab = np.zeros((HD, S), dtype=np.float64)
    for p in range(HD):
        qd, sl = p // 32, p % 32
        i = 16 * qd + (sl % 16)
        ang = t * inv_freq[i]
        ctab[p] = np.cos(ang)
        stab[p] = -np.sin(ang) if sl < 16 else np.sin(ang)
    return ctab.astype(np.float32), stab.astype(np.float32)


def _tri_mask():
    """[P, P] f32: 0 where kv(row) <= q(col) else NEG."""
    b = np.arange(P)[:, None]
    a = np.arange(P)[None, :]
    return np.where(b <= a, 0.0, NEG).astype(np.float32)


def _sel_mat():
    """[4, 4*P] bf16: sel[:, s*P:(s+1)*P] is all-zero except row s = ones."""
    m = np.zeros((4, 4 * P), dtype=np.float32)
    for s in range(4):
        m[s, s * P : (s + 1) * P] = 1.0
    return m.astype(ml_dtypes.bfloat16)


def _split8(a):
    """fp8 hi/lo split: returns (a8, ar) as float8_e4m3 arrays."""
    a = np.asarray(a, np.float32)
    a8 = a.astype(E4NP)
    ar = (a - a8.astype(np.float32)).astype(E4NP)
    return a8, ar


# ---------------------------------------------------------------------------
# device kernel
# ---------------------------------------------------------------------------
def _build_nc():
    nc = bass.Bass()

    # slot-major layouts: x slots (xr, x8); W slots (W8, Wr); Y slots (Yr, Y8)
    xp = nc.declare_dram_parameter("xp", [2 * D, S], FP8, isOutput=False)
    Wq = nc.declare_dram_parameter("Wq", [2 * D, HCOLS], FP8, isOutput=False)
    Wk = nc.declare_dram_parameter("Wk", [2 * D, HCOLS], FP8, isOutput=False)
    Wv = nc.declare_dram_parameter("Wv", [2 * D, HCOLS], FP8, isOutput=False)
    Wp = nc.declare_dram_parameter("Wp", [2 * HCOLS, D], FP8, isOutput=False)
    out = nc.declare_dram_parameter("out", [S, D], BF16, isOutput=True)

    # sqrt(SCALE)/WS on both q and k tables => scores scaled by SCALE and
    # the x32 weight pre-scale undone
    ctab_np, stab_np = _rope_tables()
    rt = np.float32(np.sqrt(SCALE) / WS)
    cq_d = nc.inline_tensor((ctab_np * rt).astype(ml_dtypes.bfloat16), "cq")
    sq_d = nc.inline_tensor((stab_np * rt).astype(ml_dtypes.bfloat16), "sq")
    mask_d = nc.inline_tensor(
        _tri_mask().astype(ml_dtypes.bfloat16), "trimask"
    )
    sel_d = nc.inline_tensor(_sel_mat(), "selmat")
    ident_d = nc.inline_tensor(np.eye(P, dtype=np.float32), "ident")
    identb_d = nc.inline_tensor(
        np.eye(P, dtype=np.float32).astype(ml_dtypes.bfloat16), "identb"
    )

    xp_t = xp[:].rearrange("(two ko p) t -> p two ko t", p=P, two=2)
    Wq_t = Wq[:].rearrange("(two ko p) m -> p two ko m", p=P, two=2)
    Wk_t = Wk[:].rearrange("(two ko p) m -> p two ko m", p=P, two=2)
    Wv_t = Wv[:].rearrange("(two ko p) m -> p two ko m", p=P, two=2)
    Wp_t = Wp[:].rearrange("(two ho p) n -> p two ho n", p=P, two=2)
    out_t = out[:].rearrange("(to p) n -> p to n", p=P)

    def mm3(ps, W_sb, x_sb, wcols, xcols, w_stationary=True):
        """3-term fp8 DoubleRow accumulation over the KT contraction tiles.

        W_sb: [P, 2, KT, *] with slots (W8, Wr); x_sb likewise with slots
        (xr, x8). wcols/xcols are free-dim slices. The stationary operand is
        W when w_stationary else x. Main terms (x8*W8, needing only the
        earliest-DMA'd slots) are issued before the cross terms.
        """
        nmm = KT // 2 + KT
        i = 0
        # main terms: k-pairs of x8 (slot 1) against W8 (slot 0)
        for ki in range(0, KT, 2):
            a = W_sb[:, 0, ki : ki + 2, wcols]
            b = x_sb[:, 1, ki : ki + 2, xcols]
            lhsT, rhs = (a, b) if w_stationary else (b, a)
            nc.tensor.matmul(ps, lhsT, rhs, start=(i == 0), stop=(i == nmm - 1),
                             perf_mode=DR)
            i += 1
        # cross terms: (W8, Wr) x (xr, x8)
        for ki in range(KT):
            a = W_sb[:, :, ki, wcols]
            b = x_sb[:, :, ki, xcols]
            lhsT, rhs = (a, b) if w_stationary else (b, a)
            nc.tensor.matmul(ps, lhsT, rhs, start=(i == 0), stop=(i == nmm - 1),
                             perf_mode=DR)
            i += 1

    with tile.TileContext(nc) as tc:
        with (
            tc.tile_pool(name="persist", bufs=1) as pp,
            tc.tile_pool(name="work", bufs=2) as wk,
        ):
            # persistent tiles
            cq = pp.tile([P, S], BF16)
            sq = pp.tile([P, S], BF16)
            trimask = pp.tile([P, P], BF16)
            identb = pp.tile([P, P], BF16)
            sel = pp.tile([4, 4 * P], BF16)
            ident = pp.tile([P, P], F32)
            ones_col = pp.tile([P, 1], BF16)
            nc.vector.memset(ones_col, 1.0)
            ones_row = pp.tile([1, P], BF16)
            nc.vector.memset(ones_row, 1.0)
            zrow = pp.tile([1, QC], BF16)
            nc.vector.memset(zrow, 0.0)

            Qt = pp.tile([P, NH_CORE, S], BF16)
            Kt = pp.tile([P, NH_CORE, S], BF16)
            Vt = pp.tile([P, TT, HCOLS], BF16)
            Yp = pp.tile([P, 2, NH_CORE, S], FP8)  # slots (Yr, Y8)

            # ------- phase 1: q/k projection + RoPE, V tiles 0..3 ----------
            # xp and Wv persist into phase 2: V tiles 4..15 are produced
            # there, interleaved into the attention stream as PE filler.
            with tc.tile_pool(name="mm1b", bufs=1) as mm1b:
                xp_sb = mm1b.tile([P, 2, KT, S], FP8)
                Wv_sb = mm1b.tile([P, 2, KT, HCOLS], FP8)

                def v_tile_mms(tt, ps):
                    """Returns the 24 matmul thunks of V tile tt (token-major;
                    x stationary), in mm3 order (mains then crosses)."""
                    tsl = slice(tt * P, (tt + 1) * P)
                    thunks = []
                    nmm = KT // 2 + KT
                    i = [0]
                    def mk(lhsT, rhs):
                        j = i[0]
                        thunks.append(lambda: nc.tensor.matmul(
                            ps, lhsT, rhs, start=(j == 0), stop=(j == nmm - 1),
                            perf_mode=DR))
                        i[0] += 1
                    for ki in range(0, KT, 2):
                        mk(xp_sb[:, 1, ki : ki + 2, tsl],
                           Wv_sb[:, 0, ki : ki + 2, :])
                    for ki in range(KT):
                        mk(xp_sb[:, :, ki, tsl], Wv_sb[:, :, ki, :])
                    return thunks

                with (
                    tc.tile_pool(name="mm1a", bufs=1) as mm1a,
                    tc.tile_pool(name="ps_mm1", bufs=8, space="PSUM") as psA,
                ):
                    Wq_sb = mm1a.tile([P, 2, KT, HCOLS], FP8)
                    Wk_sb = mm1a.tile([P, 2, KT, HCOLS], FP8)
                    # DMA priority: x8 + Wq feed the first matmuls, then xr
                    # (cross terms), then Wk, then Wv
                    for ki in range(KT):
                        nc.sync.dma_start(xp_sb[:, 1, ki], xp_t[:, 1, ki])
                        nc.gpsimd.dma_start(Wq_sb[:, 0, ki], Wq_t[:, 0, ki])
                        nc.gpsimd.dma_start(Wq_sb[:, 1, ki], Wq_t[:, 1, ki])
                    # RoPE tables: must land before the first q-tile's
                    # DVE chain (~18us) to avoid backpressure on psum bufs
                    nc.gpsimd.dma_start(cq, cq_d[:])
                    nc.gpsimd.dma_start(sq, sq_d[:])
                    for ki in range(KT):
                        nc.sync.dma_start(xp_sb[:, 0, ki], xp_t[:, 0, ki])
                        nc.gpsimd.dma_start(Wk_sb[:, 0, ki], Wk_t[:, 0, ki])
                        nc.gpsimd.dma_start(Wk_sb[:, 1, ki], Wk_t[:, 1, ki])
                    for ki in range(KT):
                        nc.gpsimd.dma_start(Wv_sb[:, 0, ki], Wv_t[:, 0, ki])
                        nc.gpsimd.dma_start(Wv_sb[:, 1, ki], Wv_t[:, 1, ki])
                    # phase-2 constants: not needed until attention
                    nc.gpsimd.dma_start(trimask, mask_d[:])
                    nc.gpsimd.dma_start(identb, identb_d[:])
                    nc.gpsimd.dma_start(sel, sel_d[:])
                    nc.gpsimd.dma_start(ident, ident_d[:])

                    # q and k projections with fused RoPE
                    for W_sb, O_t in ((Wq_sb, Qt), (Wk_sb, Kt)):
                        for h in range(NH_CORE):
                            for tcx in range(NQC):
                                ps = psA.tile([P, QC], F32, tag="ps")
                                mm3(ps, W_sb, xp_sb,
                                    slice(h * HD, (h + 1) * HD),
                                    slice(tcx * QC, (tcx + 1) * QC))
                                csl = cq[:, tcx * QC : (tcx + 1) * QC]
                                ssl = sq[:, tcx * QC : (tcx + 1) * QC]
                                pc = wk.tile([P, QC], BF16, tag="pc")
                                nc.scalar.activation(
                                    pc, ps, mybir.ActivationFunctionType.Copy
                                )
                                xsw = wk.tile([P, QC], BF16, tag="xsw")
                                nc.vector.stream_shuffle(xsw, pc, _SWAP16)
                                m1 = wk.tile([P, QC], BF16, tag="m1")
                                nc.vector.tensor_mul(m1, pc, csl)
                                m2 = wk.tile([P, QC], BF16, tag="m2")
                                nc.vector.tensor_mul(m2, xsw, ssl)
                                nc.vector.tensor_add(
                                    O_t[:, h, tcx * QC : (tcx + 1) * QC], m1, m2
                                )

                    # V tiles 0..3 (needed by the first attention q-chunk)
                    for tt in range(4):
                        ps = psA.tile([P, HCOLS], F32, tag="ps")
                        for th in v_tile_mms(tt, ps):
                            th()
                        nc.scalar.activation(
                            Vt[:, tt], ps, mybir.ActivationFunctionType.Copy,
                            scale=1.0 / WS,
                        )

                # ------ phase 2: attention + V tiles 4..15 + out proj -------
                with (
                    tc.tile_pool(name="attn", bufs=1) as atp,
                    tc.tile_pool(name="outp", bufs=4) as outp,
                    tc.tile_pool(name="pt", bufs=3) as ptp,
                    tc.tile_pool(name="ps_s", bufs=2, space="PSUM") as psS,
                    tc.tile_pool(name="ps_o", bufs=2, space="PSUM") as psO,
                    tc.tile_pool(name="ps_m", bufs=2, space="PSUM") as psM,
                    tc.tile_pool(name="ps_b", bufs=1, space="PSUM") as psB,
                    tc.tile_pool(name="ps_p", bufs=1, space="PSUM") as psP,
                ):
                    Wp_sb = atp.tile([P, 2, NH_CORE, D], FP8)
                    for s in range(2):
                        for ho in range(NH_CORE):
                            nc.sync.dma_start(Wp_sb[:, s, ho], Wp_t[:, s, ho])

                    out_n = [0]

                    def out_group(tt, ncx, pool, tag="psp"):
                        """One (tt, ncx) out-projection psum group: 6
                        DoubleRow matmuls + copy-out + DMA."""
                        ps = pool.tile([P, QC], F32, tag=tag,
                                       name=f"og{tt}_{ncx}")
                        i = 0
                        for ho in range(0, NH_CORE, 2):
                            nc.tensor.matmul(
                                ps,
                                Yp[:, 1, ho : ho + 2, tt * P : (tt + 1) * P],
                                Wp_sb[:, 0, ho : ho + 2,
                                      ncx * QC : (ncx + 1) * QC],
                                start=(i == 0),
                                stop=False,
                                perf_mode=DR,
                            )
                            i += 1
                        for ho in range(NH_CORE):
                            nc.tensor.matmul(
                                ps,
                                Yp[:, :, ho, tt * P : (tt + 1) * P],
                                Wp_sb[:, :, ho, ncx * QC : (ncx + 1) * QC],
                                start=False,
                                stop=(ho == NH_CORE - 1),
                                perf_mode=DR,
                            )
                        obp = outp.tile([P, QC], BF16, tag="obp")
                        # alternate the psum->sbuf copy between ACT and DVE
                        # to keep both below the PE roofline
                        if out_n[0] % 2 == 0:
                            nc.scalar.activation(
                                obp, ps, mybir.ActivationFunctionType.Copy
                            )
                        else:
                            nc.vector.tensor_copy(obp, ps)
                        out_n[0] += 1
                        nc.sync.dma_start(
                            out_t[:, tt, ncx * QC : (ncx + 1) * QC], obp
                        )

                    # filler fifo: 4-matmul chunks of V tiles 4..15, then
                    # out-projection groups as Y chunks complete
                    fifo = []
                    vps = {}

                    def enqueue_v(tt):
                        # psum tile created lazily at the first chunk so psP's
                        # buffer isn't claimed before the previous user's
                        # reads are issued
                        def first_chunk(thunks=None, tt=tt):
                            ps = psP.tile([P, HCOLS], F32, tag="psp",
                                          name=f"vps{tt}")
                            vps[tt] = ps
                            vthunks[tt] = v_tile_mms(tt, ps)
                            for th in vthunks[tt][0:4]:
                                th()
                        fifo.append(("v", tt, 0, first_chunk))
                        for j in range(4, 24, 4):
                            fifo.append(("v", tt, j, None))

                    vthunks = {}

                    def drain2(k, tail=False):
                        n = 0
                        while n < k and fifo:
                            item = fifo.pop(0)
                            if item[0] == "v":
                                _, tt, j, fc = item
                                if fc is not None:
                                    fc()
                                else:
                                    for th in vthunks[tt][j : j + 4]:
                                        th()
                                if j == 20:  # last chunk: copy out Vt
                                    nc.scalar.activation(
                                        Vt[:, tt],
                                        vps.pop(tt),
                                        mybir.ActivationFunctionType.Copy,
                                        scale=1.0 / WS,
                                    )
                                    del vthunks[tt]
                            else:
                                if tail:
                                    # psO/psM/psB are all free after the
                                    # final epilogue: 4-way bank rotation so
                                    # copies never block the next group
                                    pool, tag = [
                                        (psP, "psp"), (psB, "bb"),
                                        (psO, "ops"), (psM, "lm"),
                                    ][n % 4]
                                    out_group(item[1], item[2], pool, tag=tag)
                                else:
                                    out_group(item[1], item[2], psP)
                            n += 1

                    def av_step(h, qc, njb, o_ps, lm, jb, off, pt):
                        d = jb - 4 * qc
                        nc.tensor.matmul(
                            o_ps[:, off:],
                            Vt[:, jb, h * HD : (h + 1) * HD],
                            pt[:, off:],
                            start=(jb == 0),
                            stop=(jb == njb - 1),
                        )
                        # denominator: P^T tile as weights against a ones
                        # column (ap_size=1 => ~free)
                        for sub in range(max(d, 0), 4):
                            nc.tensor.matmul(
                                lm[:, sub : sub + 1],
                                pt[:, sub * P : (sub + 1) * P],
                                ones_col,
                                start=False,
                                stop=(jb == 4 * qc + sub),
                                skip_group_check=True,
                            )

                    pending = None  # deferred normalization epilogue

                    def epilogue(h, qc, o_ps, lm):
                        # l [q,4sub] -> transpose -> 1/l -> broadcast [128,q]
                        l_sb = wk.tile([P, 4], F32, tag="lsb")
                        nc.vector.tensor_copy(l_sb, lm[:, 0:4])
                        nc.tensor.transpose(lm[0:4, 4 : 4 + P], l_sb, ident)
                        rb = wk.tile([4, P], BF16, tag="rb")
                        with nc.allow_low_precision(
                            reason="softmax denom reciprocal to bf16"
                        ):
                            nc.vector.reciprocal(rb, lm[0:4, 4 : 4 + P])
                        bb = psB.tile([P, QC], F32, tag="bb")
                        # explicit zero then accumulate (safe under bank- or
                        # byte-granular psum zeroing)
                        nc.tensor.matmul(bb, ones_row, zrow, start=True,
                                         stop=False, skip_group_check=True)
                        for sub in range(4):
                            nc.tensor.matmul(
                                bb[:, sub * P : (sub + 1) * P],
                                sel[:, sub * P : (sub + 1) * P],
                                rb,
                                start=False,
                                stop=(sub == 3),
                                skip_group_check=True,
                            )
                        ym = wk.tile([P, QC], BF16, tag="ym")
                        nc.vector.tensor_mul(ym, o_ps, bb)
                        y8 = Yp[:, 1, h, qc * QC : (qc + 1) * QC]
                        nc.vector.tensor_copy(y8, ym)
                        nc.vector.tensor_sub(
                            Yp[:, 0, h, qc * QC : (qc + 1) * QC], ym, y8
                        )

                    for qc in range(NQC):
                        if qc < NQC - 1:
                            for tt in range(4 * qc + 4, 4 * qc + 8):
                                enqueue_v(tt)
                        for h in range(NH_CORE):
                            o_ps = psO.tile([P, QC], F32, tag="ops")
                            lm = psM.tile([P, QC], F32, tag="lm")
                            # zero the l columns explicitly
                            nc.tensor.matmul(lm[:, 0:4], ones_row,
                                             zrow[:, 0:4], start=True,
                                             stop=False,
                                             skip_group_check=True)
                            njb = 4 * qc + 4
                            prev = None  # software-pipelined AV/l step
                            for jb in range(njb):
                                d = jb - 4 * qc  # diag offset if >= 0
                                off = P * d if d > 0 else 0
                                drain2(2 if len(fifo) > 16 else 1)
                                s_ps = psS.tile([P, QC], F32, tag="sps")
                                nc.tensor.matmul(
                                    s_ps[:, off:],
                                    Kt[:, h, jb * P : (jb + 1) * P],
                                    Qt[:, h, qc * QC + off : (qc + 1) * QC],
                                    start=True,
                                    stop=(d < 0),
                                    skip_group_check=True,
                                )
                                if d >= 0:
                                    # causal mask via PE: += I^T @ trimask
                                    nc.tensor.matmul(
                                        s_ps[:, off : off + P],
                                        identb,
                                        trimask,
                                        start=False,
                                        stop=True,
                                        skip_group_check=True,
                                    )
                                pt = ptp.tile([P, QC], BF16, tag="pt")
                                nc.scalar.activation(
                                    pt[:, off:],
                                    s_ps[:, off:],
                                    mybir.ActivationFunctionType.Exp,
                                )
                                if prev is not None:
                                    av_step(h, qc, njb, o_ps, lm, *prev)
                                prev = (jb, off, pt)
                            av_step(h, qc, njb, o_ps, lm, *prev)
                            if pending is not None:
                                epilogue(*pending)
                                ph, pqc = pending[0], pending[1]
                                if ph == NH_CORE - 1:
                                    for tt in range(4 * pqc, 4 * pqc + 4):
                                        for ncx in range(D // QC):
                                            fifo.append(("o", tt, ncx))
                            pending = (h, qc, o_ps, lm)
                    epilogue(*pending)
                    for tt in range(4 * (NQC - 1), TT):
                        for ncx in range(D // QC):
                            fifo.append(("o", tt, ncx))
                    # tail: alternate psum banks (psB is free after the final
                    # epilogue) to pipeline group vs copy
                    drain2(len(fifo), tail=True)
    return nc


# ---------------------------------------------------------------------------
# legalization: this walrus build supports only ONE sync wait per instruction
# ---------------------------------------------------------------------------
_ENGINE_SEM_PREFIX = {
    "PE": "PE_",
    "DVE": "DVE_",
    "ACT": "ACT_",
    "Pool": "POOL_",
    "SP": "SP_",
}
_wf_counter = [0]


def _legalize(nc, max_waits=1):
    for f in nc.m.functions:
        for bb in f.blocks:
            new_insts = []
            for inst in bb.instructions:
                si = getattr(inst, "sync_info", None)
                eng = getattr(inst, "engine", None)
                if si is None or not si.on_wait or eng is None:
                    new_insts.append(inst)
                    continue
                waits = list(si.on_wait)
                pref = _ENGINE_SEM_PREFIX.get(eng.name)
                if pref is not None:
                    waits = [
                        w
                        for w in waits
                        if not (
                            w.sync_type == "semaphore"
                            and w.ant_name.startswith(pref)
                        )
                    ]
                if len(waits) > max_waits:
                    for w in waits[:-max_waits]:
                        _wf_counter[0] += 1
                        nop = mybir.InstNoOp(
                            name=f"I-waitfix-{_wf_counter[0]}", ins=[], outs=[]
                        )
                        nop.engine = eng
                        nop.sync_info = mybir.SyncInfo(on_wait=[w], on_update=[])
                        new_insts.append(nop)
                    waits = waits[-max_waits:]
                if len(waits) != len(si.on_wait):
                    inst.sync_info = mybir.SyncInfo(
                        on_wait=waits, on_update=list(si.on_update)
                    )
                new_insts.append(inst)
            bb.instructions[:] = new_insts


# ---------------------------------------------------------------------------
# SPMD runner (mirrors concourse.bass2jax.run_bass_via_pjrt, kept resident)
# ---------------------------------------------------------------------------
class _Runner:
    def __init__(self, nc, n_cores=8):
        import jax
        from jax.sharding import Mesh, PartitionSpec
        from jax.experimental.shard_map import shard_map
        from concourse import bass2jax
        from concourse.bass2jax import _bass_exec_p, install_neuronx_cc_hook

        install_neuronx_cc_hook()
        self.jax = jax
        self.nc = nc
        self.n_cores = n_cores
        partition_name = (
            nc.partition_id_tensor.name if nc.partition_id_tensor else None
        )
        in_names, out_names, out_avals, zero_outs = [], [], [], []
        for alloc in nc.m.functions[0].allocations:
            if not isinstance(alloc, mybir.MemoryLocationSet):
                continue
            name = alloc.memorylocations[0].name
            if alloc.kind == "ExternalInput":
                if name != partition_name:
                    in_names.append(name)
            elif alloc.kind == "ExternalOutput":
                shape = tuple(alloc.tensor_shape)
                dtype = mybir.dt.np(alloc.dtype)
                out_names.append(name)
                out_avals.append(jax.core.ShapedArray(shape, dtype))
                zero_outs.append(np.zeros(shape, dtype))
        self.in_names, self.out_names = in_names, out_names
        self.out_avals, self.zero_outs = out_avals, zero_outs
        n_params, n_outs = len(in_names), len(out_names)
        all_in_names = in_names + out_names
        if partition_name is not None:
            all_in_names.append(partition_name)
        donate = tuple(range(n_params, n_params + n_outs))

        def _body(*args):
            operands = list(args)
            if partition_name is not None:
                operands.append(bass2jax.partition_id_tensor())
            return tuple(
                _bass_exec_p.bind(
                    *operands,
                    out_avals=tuple(out_avals),
                    in_names=tuple(all_in_names),
                    out_names=tuple(out_names),
                    lowering_input_output_aliases=(),
                    sim_require_finite=True,
                    sim_require_nnan=True,
                    nc=nc,
                )
            )

        devices = jax.devices()[:n_cores]
        mesh = Mesh(np.asarray(devices), ("core",))
        in_specs = (PartitionSpec("core"),) * (n_params + n_outs)
        out_specs = (PartitionSpec("core"),) * n_outs
        self.fn = jax.jit(
            shard_map(
                _body,
                mesh=mesh,
                in_specs=in_specs,
                out_specs=out_specs,
                check_rep=False,
            ),
            donate_argnums=donate,
            keep_unused=True,
        )

    def run(self, in_maps):
        n = self.n_cores
        concat_in = [
            np.concatenate(
                [np.asarray(in_maps[c][name]) for c in range(n)], axis=0
            )
            for name in self.in_names
        ]
        zeros = [
            np.zeros((n * z.shape[0], *z.shape[1:]), z.dtype)
            for z in self.zero_outs
        ]
        out_arrs = self.fn(*concat_in, *zeros)
        return [
            {
                name: np.asarray(out_arrs[i]).reshape(
                    n, *self.out_avals[i].shape
                )[c]
                for i, name in enumerate(self.out_names)
            }
            for c in range(n)
        ]


_RUNNER = None


def _get_runner():
    global _RUNNER
    if _RUNNER is None:
        nc = _build_nc()
        _legalize(nc)
        _RUNNER = _Runner(nc, 8)
    return _RUNNER


# ---------------------------------------------------------------------------
# input prep (host-side sharding + fp8 decomposition)
# ---------------------------------------------------------------------------
def _prep_in_maps(x, Wqkv, Wproj):
    x = np.asarray(x, dtype=np.float32)
    Wqkv = np.asarray(Wqkv, dtype=np.float32)
    Wproj = np.asarray(Wproj, dtype=np.float32)
    perm = _dim_perm()

    xps = []
    for b in range(B):
        xT = np.ascontiguousarray(x[b].T)
        x8, xr = _split8(xT)
        # [2*D, S] slot-major: slot 0 = xr, slot 1 = x8
        xps.append(np.concatenate([xr, x8], axis=0))

    def wsplit(Wm):  # [K, N] -> [2*K, N] slot-major (W8, Wr)
        w8, wr = _split8(Wm * WS)
        return np.concatenate([w8, wr], axis=0)

    in_maps = []
    for c in range(8):
        b, g = c // 4, c % 4
        heads = range(NH_CORE * g, NH_CORE * (g + 1))
        qcols = np.concatenate([h * HD + perm for h in heads])
        in_maps.append(
            {
                "xp": xps[b],
                "Wq": wsplit(Wqkv[:, 0 * D + qcols]),
                "Wk": wsplit(Wqkv[:, 1 * D + qcols]),
                "Wv": wsplit(
                    Wqkv[:, 2 * D + g * HCOLS : 2 * D + (g + 1) * HCOLS]
                ),
                "Wp": wsplit(Wproj[g * HCOLS : (g + 1) * HCOLS, :]),
            }
        )
    return in_maps


# ---------------------------------------------------------------------------
# public entry point
# ---------------------------------------------------------------------------
def kernel(x, Wqkv, Wproj):
    in_maps = _prep_in_maps(x, Wqkv, Wproj)
    results = _get_runner().run(in_maps)
    out = np.zeros((B, S, D), dtype=np.float32)
    for c in range(8):
        out[c // 4] += results[c]["out"].astype(np.float32)
    out *= np.float32(1.0 / WS)  # undo the Wp x32 pre-scale (device keeps x32)
    return out
